# revision 2
# baseline (speedup 1.0000x reference)
"""Band-sparse (local block) attention on 8 TRN2 NeuronCores.

Problem: q,k,v [4096, 8, 64] f32; block size 128; banded block mask with 4
blocks each side of the diagonal (window 512). pair_bias is unused.

Sharding: one head per NeuronCore (8 heads / 8 cores). Each core computes
its head's banded attention; host slices/transposes inputs and reassembles
the output.

Per-core algorithm (head h), v2:
  Layout: qt [128, 4096] bf16 = q^T pre-scaled by 16*log2(e), duplicated
          into partitions 64..127; kt [128, 4096] likewise duplicated;
          vo [128, 32, 65] = per key block j-major V plus a ones column.
  QK is row-tiled: even key blocks run on PE array rows 0..63
  (tile_position (0,0)), odd blocks on rows 64..127 ((64,0)) — the d=64
  contraction only needs half the array, so two blocks stream
  concurrently (2x QK throughput). MM emission interleaves the two
  streams so the hardware overlaps them.
  Scores (pre-scaled: U = 128*log2 e^(s/8)) land in PSUM f32. The exp is
  split across two engines, chosen per key block to balance load:
    - ScalarE ACT: p = exp(U * ln2/128) -> bf16
    - VectorE custom op EXP_BITS_ANT: computes the bf16 BIT PATTERN of
      ~exp(s/8) as int16 in one instruction (magic-number round trick +
      quadratic correction; centered log err ~ +-0.55%, mean ~0).
  PV accumulates o_ps[65, 512] per 4-block query group in PSUM over the
  12 intersecting key blocks (ones row = softmax denominator).
  Evacuate o_ps via ScalarE copy, DMA out as ot [65, 4096] f32.
Host: out = (ot[:64] / ot[64:65]).T per head.
"""

import os
import sys

import numpy as np


def _ensure_path():
    try:
        import concourse  # noqa: F401
    except ImportError:
        for p in ("/opt/trn_rl_repo", "/root/.axon_site/_ro/trn_rl_repo"):
            if os.path.isdir(p) and p not in sys.path:
                sys.path.insert(0, p)


_ensure_path()

import ml_dtypes  # noqa: E402

import concourse.bacc as bacc  # noqa: E402
import concourse.tile as tile  # noqa: E402
from concourse import mybir  # noqa: E402
from concourse.bass_utils import run_bass_kernel_spmd  # noqa: E402

N, H, D, B = 4096, 8, 64, 128
NROW = N // B  # 32 row/key blocks
BPS = 4  # band: blocks per side
F32 = mybir.dt.float32
BF16 = mybir.dt.bfloat16
I16 = mybir.dt.int16
NP_BF16 = ml_dtypes.bfloat16
MAXW = (2 * BPS + 1) * B  # 1152: widest band span

# exp-op constants. Scores are pre-scaled on the host (q *= C0SCALE) so the
# PSUM value is U = 128 * log2(exp(s/8)). ACT recovers exp(s/8) with its
# free affine scale; the DVE op consumes U directly.
C0SCALE = float(16 * np.log2(np.e))
ACT_SCALE = float(np.log(2.0) / 128.0)
CN = -70.1359130
CS = 0.00228512688
C1MAGIC = float(np.float32(1.5 * 2**30 + 16256))
CBX = 16244.620445507204


def _register_exp_bits():
    """Register the EXP_BITS_ANT custom DVE op (idempotent).

    out_i16 = round_to_int16( U + CS*(|U - K| + CN)^2 + CBX ),
    K = (U + C1MAGIC) - C1MAGIC = 128*round(U/128) exactly.
    The int16 is the bf16 bit pattern of ~exp(s/8) (up to a constant
    factor that cancels in the softmax ratio; CBX zeroes the mean log
    error so ACT- and DVE-computed blocks agree in absolute scale).
    """
    import concourse.dve_ops as dve_ops
    from concourse.dve_spec import (
        Spec,
        Src0,
        C0,
        C1,
        C2,
        C3,
        Bin,
        AluOp,
        lower,
        sq,
        _spill_c3_to_src1,
        _has_src1,
    )
    from concourse.dve_uop import DveOpSpec

    name = "EXP_BITS_ANT"
    for op in dve_ops.OPS:
        if op.name == name:
            return op

    t = Src0 + C1
    k3 = t - C1
    a = Bin(AluOp.ABSOLUTE_DIFF, Src0, k3)
    n1 = a + C0
    n2 = sq(n1)
    n3 = n2 * C2
    body = _spill_c3_to_src1((Src0 + n3) + C3)

    def _np_reference(in0, in1, s0, s1, imm2):
        f32 = np.float32
        U = in0.astype(f32)
        tt = f32(U + f32(s1))
        kk = f32(tt - f32(s1))
        aa = f32(np.abs(f32(U - kk)))
        nn1 = f32(aa + f32(s0))
        nn3 = f32(f32(nn1 * nn1) * f32(imm2))
        return f32(f32(U + nn3) + in1)

    spec = Spec(body=body, reference=_np_reference)
    opcode = dve_ops._CUSTOM_DVE_ROW_BASE + len(dve_ops.OPS)
    shas = {}
    for ver in ("v3", "v4"):
        s = DveOpSpec(
            name=name, opcode=opcode, uops=lower(spec, ver=ver),
            rd1_en=_has_src1(spec),
        )
        shas[ver] = s.sha(ver)
    op = dve_ops.DveOp(name, spec, subdim=False, uops_sha=shas)
    dve_ops._SUB_OPCODE_FOR_NAME[name] = opcode
    dve_ops.OPS.append(op)
    dve_ops.CUSTOM_DVE_SPECS[name] = spec
    return op


EXP_BITS = _register_exp_bits()


def _band(c):
    """Valid query-block range for key block c (inclusive)."""
    return max(0, c - BPS), min(NROW - 1, c + BPS)


def _width(c):
    r_lo, r_hi = _band(c)
    return (r_hi - r_lo + 1) * B


# exp-engine assignment: DVE gets the odd blocks plus block 0 (so the
# ScalarE ACT table load overlaps the start instead of gating exp(0)).
DVE_BLOCKS = {0} | {c for c in range(NROW) if c % 2 == 1}


def _build_nc():
    nc = bacc.Bacc(None)
    qt_d = nc.dram_tensor("qt", [2 * D, N], BF16, kind="ExternalInput")
    kt_d = nc.dram_tensor("kt", [2 * D, N], BF16, kind="ExternalInput")
    vo_d = nc.dram_tensor("vo", [B, NROW, D + 1], BF16, kind="ExternalInput")
    ot_d = nc.dram_tensor("ot", [D + 1, N], F32, kind="ExternalOutput")

    with tile.TileContext(nc) as tc:
        with (
            tc.tile_pool(name="io", bufs=1) as io_pool,
            tc.tile_pool(name="pexp", bufs=6) as p_pool,
            tc.tile_pool(name="pint", bufs=6) as pi_pool,
            tc.tile_pool(name="st", bufs=2, space="PSUM") as st_pool,
            tc.tile_pool(name="acc", bufs=2, space="PSUM") as acc_pool,
            tc.tile_pool(name="ev", bufs=2) as ev_pool,
        ):
            qt = io_pool.tile([2 * D, N], BF16)
            kt = io_pool.tile([2 * D, N], BF16)
            vo = io_pool.tile([B, NROW, D + 1], BF16)
            cbx = io_pool.tile([B, 1], F32)
            wz = io_pool.tile([B, 512], BF16)

            # Input DMAs first, split across the sync and gpsimd queues so
            # the two tensors stream in parallel and the first chunks land
            # ~1.5us in. vo is needed from ~3us (first PV).
            CH = N // 4
            nc.vector.memset(cbx, CBX)
            nc.gpsimd.memset(wz, 0.0)
            for i, which in enumerate(("kt", "qt", "kt", "qt")):
                cs = slice(i * CH, (i + 1) * CH)
                src = kt_d if which == "kt" else qt_d
                dst = kt if which == "kt" else qt
                nc.sync.dma_start(out=dst[:, cs], in_=src[:, cs])
            gp_order = [("qt", 0), ("vo", 0), ("kt", 1), ("vo", 1),
                        ("qt", 2), ("kt", 3), ("vo", 2), ("vo", 3)]
            for which, i in gp_order:
                if which == "vo":
                    bs = slice(i * (NROW // 4), (i + 1) * (NROW // 4))
                    nc.gpsimd.dma_start(out=vo[:, bs, :], in_=vo_d[:, bs, :])
                else:
                    cs = slice(i * CH, (i + 1) * CH)
                    src = kt_d if which == "kt" else qt_d
                    dst = kt if which == "kt" else qt
                    nc.gpsimd.dma_start(out=dst[:, cs], in_=src[:, cs])

            # HAM warmup: keep the PE busy from ~0.6us until the real QK
            # stream starts (~2us). 4 cold 512-col matmuls ~ 1.7us.
            wps = acc_pool.tile([B, 512], F32, name="wps", tag="ops")
            for _ in range(4):
                nc.tensor.matmul(wps[:, :], wz[:, :B], wz[:, :], start=True,
                                 stop=True)

            P = {}  # c -> (bf16-view AP source tile, q_lo)
            o_ps = {}

            def qk(c):
                """Emit the interleaved-pair QK matmuls for block c's
                stream half, plus its exp. Returns list of (out,lhsT,rhs)
                matmul args instead of emitting when paired."""
                r_lo, r_hi = _band(c)
                q_lo = r_lo * B
                w = (r_hi - r_lo + 1) * B
                half = slice(0, D) if c % 2 == 0 else slice(D, 2 * D)
                st = st_pool.tile([B, MAXW], F32, tag="st")
                mms = []
                for off in range(0, w, 512):
                    n = min(512, w - off)
                    mms.append((
                        st[:, off:off + n],
                        kt[half, c * B:(c + 1) * B],
                        qt[half, q_lo + off:q_lo + off + n],
                    ))
                return st, q_lo, w, mms

            def emit_exp(c, st, q_lo, w):
                if c in DVE_BLOCKS:
                    pi = pi_pool.tile([B, MAXW], I16, tag="pi")
                    nc.vector._custom_dve(
                        EXP_BITS, out=pi[:, :w], in0=st[:, :w],
                        in1=cbx[:, 0:1], s0=CN, s1=C1MAGIC, imm2=CS,
                    )
                    P[c] = (pi.bitcast(BF16), q_lo)
                else:
                    pc = p_pool.tile([B, MAXW], BF16, tag="pc")
                    nc.scalar.activation(
                        pc[:, :w], st[:, :w],
                        mybir.ActivationFunctionType.Exp, scale=ACT_SCALE,
                    )
                    P[c] = (pc, q_lo)

            def qk_pair(c0, c1):
                st0, ql0, w0, mm0 = qk(c0)
                st1, ql1, w1, mm1 = qk(c1)
                for i in range(max(len(mm0), len(mm1))):
                    if i < len(mm0):
                        nc.tensor.matmul(*mm0[i], start=True, stop=True)
                    if i < len(mm1):
                        nc.tensor.matmul(*mm1[i], start=True, stop=True)
                emit_exp(c0, st0, ql0, w0)
                emit_exp(c1, st1, ql1, w1)

            def pv(g, c, first_call, last_call):
                # accumulate key block c's contribution to query group g.
                # PSUM group semantics: start=True once per accumulator bank
                # (first matmul), stop=True on the very last matmul into the
                # bank. Rows split into runs by "is this row's first
                # contribution" so each matmul's bytes are uniformly fresh
                # or accumulating.
                r_lo = max(4 * g, c - BPS, 0)
                r_hi = min(4 * g + 3, c + BPS, NROW - 1)
                if r_lo > r_hi:
                    return
                pc, q_lo = P[c]
                runs = []
                for r in range(r_lo, r_hi + 1):
                    fresh = c == max(0, r - BPS)
                    if runs and runs[-1][2] == fresh:
                        runs[-1][1] = r
                    else:
                        runs.append([r, r, fresh])
                for i, (ra, rb, _fresh) in enumerate(runs):
                    nc.tensor.matmul(
                        o_ps[g][:, (ra - 4 * g) * B:(rb + 1 - 4 * g) * B],
                        vo[:, c, :],
                        pc[:, ra * B - q_lo:(rb + 1) * B - q_lo],
                        start=first_call and i == 0,
                        stop=last_call and i == len(runs) - 1,
                    )

            def evac(g):
                ev = ev_pool.tile([D + 1, 4 * B], F32, tag="ev")
                nc.scalar.copy(ev[:, :], o_ps[g][:, :])
                nc.sync.dma_start(
                    out=ot_d[:, 4 * g * B:(4 * g + 4) * B], in_=ev[:, :]
                )

            def pv_step(step):
                # baseline bookkeeping: at `step`, PV-consume block step-1.
                for g in range(NROW // 4):
                    s0 = 4 * g + 1
                    c_first = max(0, 4 * g - BPS)
                    c_last = min(NROW - 1, 4 * g + BPS + 3)
                    if step == s0:
                        o_ps[g] = acc_pool.tile(
                            [D + 1, 4 * B], F32, name="ops", tag="ops"
                        )
                        for cc in range(c_first, s0):
                            pv(g, cc, cc == c_first, cc == c_last)
                    elif s0 < step <= 4 * g + BPS + 4:
                        c = step - 1
                        pv(g, c, c == c_first, c == c_last)
                    if step == c_last + 1:
                        evac(g)

            for pair in range(NROW // 2 + 1):
                if pair < NROW // 2:
                    qk_pair(2 * pair, 2 * pair + 1)
                for sub in (0, 1):
                    step = 2 * pair + sub
                    if step <= NROW:
                        pv_step(step)

    nc.compile()
    return nc


_NC = None


def _get_nc():
    global _NC
    if _NC is None:
        _NC = _build_nc()
    return _NC


def _make_in_maps(q, k, v):
    q = np.ascontiguousarray(q, dtype=np.float32)
    k = np.ascontiguousarray(k, dtype=np.float32)
    v = np.ascontiguousarray(v, dtype=np.float32)
    in_maps = []
    for h in range(H):
        qT = (q[:, h, :].T * np.float32(C0SCALE)).astype(NP_BF16)  # [64, N]
        kT = k[:, h, :].T.astype(NP_BF16)
        qT2 = np.ascontiguousarray(np.concatenate([qT, qT], axis=0))
        kT2 = np.ascontiguousarray(np.concatenate([kT, kT], axis=0))
        vb = v[:, h, :].reshape(NROW, B, D).transpose(1, 0, 2)  # [128, 32, 64]
        vo = np.concatenate(
            [vb, np.ones((B, NROW, 1), np.float32)], axis=2
        ).astype(NP_BF16)  # [128, 32, 65]
        in_maps.append(
            {"qt": qT2, "kt": kT2, "vo": np.ascontiguousarray(vo)}
        )
    return in_maps


def run(q, k, v, trace=False, **trace_kwargs):
    """Returns (out [4096, 8, 64] f32, BassKernelResults)."""
    nc = _get_nc()
    in_maps = _make_in_maps(q, k, v)
    res = run_bass_kernel_spmd(
        nc, in_maps, list(range(H)), trace=trace, **trace_kwargs
    )
    out = np.empty((N, H, D), dtype=np.float32)
    for h in range(H):
        ot = res.results[h]["ot"]  # [65, 4096]
        out[:, h, :] = (ot[:D] / ot[D:D + 1]).T
    return out, res


def kernel(q, k, v, pair_bias=None):
    out, _ = run(q, k, v)
    return out
